# Initial kernel scaffold
#
"""Trainium2 Bass kernel for nn_Autoencoder_65120294142543 (ECT autoencoder).

Sharding (8 NeuronCores, one TRN2 chip):
  - ECT layers: data-parallel over graphs (32 graphs = 3200 nodes per core).
  - MLP: tensor-parallel. W1 column-sharded [4096, 512/core] (bf16),
    W2 row-sharded [512/core, 4096] (bf16), W3 replicated (bf16).
    AllGather of the (augmented) ECT output before L1; ReduceScatter of the
    L2 partial sums (by graph) after L2.
  - Normalization e/max(e) is folded into the MLP via an augmented
    contraction row: Z1 = e@W1 + mx*b1, h1 = tanh(Z1 * (1/mx)).
"""

import numpy as np

# ---------------- problem constants (hardcoded per the task spec) ----------
B = 256          # graphs
NPG = 100        # nodes per graph
T = 64           # num directions (thetas)
J = 64           # bump steps (thresholds)
D = J * T        # 4096 = flattened ECT size = MLP input dim
HID = 4096
NCORES = 8
GPC = B // NCORES          # 32 graphs per core
NPC = GPC * NPG            # 3200 nodes per core
NT = NPC // 128            # 25 node tiles of 128
JGS = 8                    # js per sigma/segsum group
NJG = J // JGS             # 8 groups
WCOL = HID // NCORES       # 512 W1 columns / W2 rows per core
SCALE = 500.0
NB2 = HID // 512           # 8 n-blocks for L2 output

_CACHE = {}


def _patch_tile_drain():
    """The walrus build deployed here supports fewer sem-waits per CTRL
    instruction than Tile's kernel-tail drain accumulates.  Split the
    drain's waits into individual wait_ge instructions."""
    from concourse import tile
    from concourse.tile import ScopedClock

    if getattr(tile.TileContext, "_drain_patched", False):
        return

    def _drain_and_barrier(self, tick_clock, wait_clock):
        drain_inst = self.nc.sync.drain()
        wait_clock.add_sem_waits(
            drain_inst.ins, ScopedClock({None: tick_clock.global_clock})
        )
        si = drain_inst.ins.sync_info
        if si is not None and si.on_wait and len(si.on_wait) > 1:
            waits = list(si.on_wait)
            si.on_wait = []
            by_name = {h.name: h for h in self.sems.allocated().values()}
            for w in waits:
                self.nc.sync.wait_ge(by_name[w.ant_name], w.wait_value)
        self.nc.all_engine_barrier()
        popped = self.nc._tile_sem_poison_stack.pop()
        assert popped is self._sem_poison
        self.nc.clear_and_free_semaphores(list(self.sems.allocated().values()))
        self.nc.all_engine_barrier()

    tile.TileContext._drain_and_barrier = _drain_and_barrier
    tile.TileContext._drain_patched = True


def _build(stage="full"):
    from contextlib import ExitStack
    from concourse import bass, tile, mybir

    _patch_tile_drain()

    F32 = mybir.dt.float32
    BF16 = mybir.dt.bfloat16
    AF = mybir.ActivationFunctionType
    AX = mybir.AxisListType
    ALU = mybir.AluOpType

    nc = bass.Bass(target_bir_lowering=False)

    # ---- per-core external inputs -----------------------------------------
    xT_p = nc.dram_tensor("xt", [2, NPC], F32, kind="ExternalInput")
    v_p = nc.dram_tensor("v", [2, T], F32, kind="ExternalInput")
    bias_p = nc.dram_tensor("biasrep", [128, J], F32, kind="ExternalInput")
    s1_p = nc.dram_tensor("s1", [NT, 128, GPC], BF16, kind="ExternalInput")
    s2_p = nc.dram_tensor("s2", [NT, 128, GPC], BF16, kind="ExternalInput")
    w1_p = nc.dram_tensor("w1s", [HID + 1, WCOL], BF16, kind="ExternalInput")
    w2_p = nc.dram_tensor("w2s", [WCOL + 1, HID], BF16, kind="ExternalInput")
    w3_p = nc.dram_tensor("w3a", [HID + 1, 256], BF16, kind="ExternalInput")
    idf_p = nc.dram_tensor("idf32", [128, 128], F32, kind="ExternalInput")
    idb_p = nc.dram_tensor("idbf16", [128, 128], BF16, kind="ExternalInput")

    # ---- per-core external outputs ----------------------------------------
    dec_p = nc.dram_tensor("decoded", [GPC, D], F32, kind="ExternalOutput")
    pts_p = nc.dram_tensor("pts", [NPC, 2], F32, kind="ExternalOutput")
    dbg_p = None
    if stage == "ect1":
        dbg_p = nc.dram_tensor("dbg_e", [GPC, D + 8], F32, kind="ExternalOutput")

    # ---- internal DRAM (collective bounce buffers) ------------------------
    ag_in = nc.dram_tensor("ag_in", [GPC, D + 8], F32)
    ag_out = nc.dram_tensor("ag_out", [B, D + 8], F32, addr_space="Shared")
    z2_dram = nc.dram_tensor("z2part", [B, HID], F32)
    z2own = nc.dram_tensor("z2own", [GPC, HID], F32, addr_space="Shared")

    RG = [list(range(NCORES))]

    with ExitStack() as ctx:
        tc = ctx.enter_context(tile.TileContext(nc))
        const = ctx.enter_context(tc.tile_pool(name="const", bufs=1))
        work = ctx.enter_context(tc.tile_pool(name="work", bufs=2))
        sigp = ctx.enter_context(tc.tile_pool(name="sigp", bufs=2))
        small = ctx.enter_context(tc.tile_pool(name="small", bufs=2))
        dramp = ctx.enter_context(tc.tile_pool(name="dramp", bufs=1, space="DRAM"))
        psnh = ctx.enter_context(tc.tile_pool(name="psnh", bufs=2, space="PSUM"))
        pse = ctx.enter_context(tc.tile_pool(name="pse", bufs=2, space="PSUM"))
        pstr = ctx.enter_context(tc.tile_pool(name="pstr", bufs=2, space="PSUM"))
        psmm = ctx.enter_context(tc.tile_pool(name="psmm", bufs=2, space="PSUM"))

        # ---------- constants into SBUF (DMA'd once; overlap with compute)
        v_sb = const.tile([2, T], F32)
        nc.sync.dma_start(v_sb[:], v_p[:, :])
        bias_sb = const.tile([128, J], F32)
        nc.sync.dma_start(bias_sb[:], bias_p[:, :])
        s1_sb = const.tile([128, NT, GPC], BF16)
        nc.sync.dma_start(s1_sb[:], s1_p[:, :, :].transpose([1, 0, 2]))
        s2_sb = const.tile([128, NT, GPC], BF16)
        nc.sync.dma_start(s2_sb[:], s2_p[:, :, :].transpose([1, 0, 2]))
        idf_sb = const.tile([128, 128], F32)
        nc.sync.dma_start(idf_sb[:], idf_p[:, :])
        idb_sb = const.tile([128, 128], BF16)
        nc.sync.dma_start(idb_sb[:], idb_p[:, :])
        ones_sb = const.tile([1, 128], BF16)
        nc.vector.memset(ones_sb[:], 1.0)

        w1_sb = const.tile([128, HID // 128, WCOL], BF16)
        nc.sync.dma_start(
            w1_sb[:], w1_p[0:HID, :].rearrange("(a p) n -> p a n", p=128)
        )
        w1a_sb = const.tile([1, WCOL], BF16)
        nc.sync.dma_start(w1a_sb[:], w1_p[HID : HID + 1, :])

        w2_sb = const.tile([128, WCOL // 128, HID], BF16)
        nc.sync.dma_start(
            w2_sb[:], w2_p[0:WCOL, :].rearrange("(a p) n -> p a n", p=128)
        )
        w2a_sb = const.tile([1, HID], BF16)
        nc.sync.dma_start(w2a_sb[:], w2_p[WCOL : WCOL + 1, :])

        w3_sb = const.tile([128, HID // 128, 256], BF16)
        nc.sync.dma_start(
            w3_sb[:], w3_p[0:HID, :].rearrange("(a p) n -> p a n", p=128)
        )
        w3a_sb = const.tile([1, 256], BF16)
        nc.sync.dma_start(w3a_sb[:], w3_p[HID : HID + 1, :])

        xT_sb = const.tile([2, NPC], F32)
        nc.sync.dma_start(xT_sb[:], xT_p[:, :])

        # ---------- one ECT layer: heights -> sigmoids -> per-graph sums ---
        def ect_layer(srcT_sb, S_sb, tag):
            # nh[node, t] = src[node, :] @ V   (nodes on partitions, tiled)
            nh_sb = work.tile([128, NT, T], F32, tag="nh")
            for tau in range(NT):
                pm = psnh.tile([128, T], F32, tag="psnh")
                nc.tensor.matmul(
                    pm[:],
                    lhsT=srcT_sb[:, tau * 128 : (tau + 1) * 128],
                    rhs=v_sb[:],
                    start=True,
                    stop=True,
                )
                nc.vector.tensor_copy(nh_sb[:, tau, :], pm[:])

            # e[g, j*64+t] accumulated in groups of JGS thresholds
            e_sb = work.tile([GPC, D + 8], F32, tag="e")
            nc.vector.memset(e_sb[:, D : D + 8], 0.0)
            for jg in range(NJG):
                sig = sigp.tile([128, NT, JGS, T], BF16, tag="sig")
                for jj in range(JGS):
                    j = jg * JGS + jj
                    nc.scalar.activation(
                        sig[:, :, jj, :],
                        nh_sb[:, :, :],
                        AF.Sigmoid,
                        bias=bias_sb[:, j : j + 1],
                        scale=-SCALE,
                    )
                pe = pse.tile([GPC, JGS * T], F32, tag="pse")
                for tau in range(NT):
                    nc.tensor.matmul(
                        pe[:],
                        lhsT=S_sb[:, tau, :],
                        rhs=sig[:, tau, :, :].rearrange("p a b -> p (a b)"),
                        start=(tau == 0),
                        stop=(tau == NT - 1),
                    )
                nc.vector.tensor_copy(
                    e_sb[:, jg * JGS * T : (jg + 1) * JGS * T], pe[:]
                )
            # per-graph max -> augmented column D
            mx = small.tile([GPC, 1], F32, tag="mx")
            nc.vector.reduce_max(mx[:], e_sb[:, 0:D], axis=AX.X)
            nc.vector.tensor_copy(e_sb[:, D : D + 1], mx[:])
            return e_sb, mx

        # ================= ECT layer 1 =====================================
        e1_sb, _mx1 = ect_layer(xT_sb, s1_sb, "ect1")
        nc.sync.dma_start(ag_in[:, :], e1_sb[:])

        if stage == "ect1":
            nc.sync.dma_start(dbg_p[:, :], e1_sb[:])
            return nc

        nc.gpsimd.collective_compute(
            "AllGather",
            ALU.bypass,
            ins=[ag_in[:, :]],
            outs=[ag_out[:, :]],
            replica_groups=RG,
        )

        # ================= MLP L1 (tensor-parallel over W1 columns) ========
        # lhsT chunks: transpose e_full [256, 4096] -> eT [4096, 256] in bf16
        h1_sb = []  # per g-block [128, WCOL] bf16
        for gb in range(2):
            gsl = slice(gb * 128, (gb + 1) * 128)
            z1 = psmm.tile([128, WCOL], F32, tag="z1")
            for kc in range(HID // 128):
                eg = small.tile([128, 128], F32, tag="eg")
                nc.sync.dma_start(
                    eg[:], ag_out[gsl, kc * 128 : (kc + 1) * 128]
                )
                pt = pstr.tile([128, 128], F32, tag="ptr")
                nc.tensor.transpose(pt[:], eg[:], idf_sb[:])
                eT = small.tile([128, 128], BF16, tag="eT")
                nc.vector.tensor_copy(eT[:], pt[:])
                nc.tensor.matmul(
                    z1[:],
                    lhsT=eT[:],
                    rhs=w1_sb[:, kc, :],
                    start=(kc == 0),
                    stop=False,
                )
            # augmented row: mx (per graph) * b1
            mxT = small.tile([1, 128], F32, tag="mxT")
            nc.sync.dma_start(
                mxT[:], ag_out[gsl, D : D + 1].transpose([1, 0])
            )
            mxTb = small.tile([1, 128], BF16, tag="mxTb")
            nc.vector.tensor_copy(mxTb[:], mxT[:])
            nc.tensor.matmul(
                z1[:], lhsT=mxTb[:], rhs=w1a_sb[:], start=False, stop=True
            )
            # h1 = tanh(z1 / mx)
            mxg = small.tile([128, 1], F32, tag="mxg")
            nc.sync.dma_start(mxg[:], ag_out[gsl, D : D + 1])
            rinv = small.tile([128, 1], F32, tag="rinv")
            nc.vector.reciprocal(rinv[:], mxg[:])
            h1 = work.tile([128, WCOL], BF16, tag="h1")
            nc.scalar.activation(h1[:], z1[:], AF.Tanh, scale=rinv[:])
            h1_sb.append(h1)

        # ================= MLP L2 (row-sharded W2, partial sums) ===========
        for gb in range(2):
            h1T = work.tile([128, WCOL // 128, 128], BF16, tag="h1T")
            for kc in range(WCOL // 128):
                pt = pstr.tile([128, 128], F32, tag="ptr")
                nc.tensor.transpose(
                    pt[:], h1_sb[gb][:, kc * 128 : (kc + 1) * 128], idb_sb[:]
                )
                nc.vector.tensor_copy(h1T[:, kc, :], pt[:])
            for nb in range(NB2):
                z2 = psmm.tile([128, 512], F32, tag="z2")
                for kc in range(WCOL // 128):
                    nc.tensor.matmul(
                        z2[:],
                        lhsT=h1T[:, kc, :],
                        rhs=w2_sb[:, kc, nb * 512 : (nb + 1) * 512],
                        start=(kc == 0),
                        stop=False,
                    )
                nc.tensor.matmul(
                    z2[:],
                    lhsT=ones_sb[:],
                    rhs=w2a_sb[:, nb * 512 : (nb + 1) * 512],
                    start=False,
                    stop=True,
                )
                z2st = small.tile([128, 512], F32, tag="z2st")
                nc.vector.tensor_copy(z2st[:], z2[:])
                nc.sync.dma_start(
                    z2_dram[gb * 128 : (gb + 1) * 128, nb * 512 : (nb + 1) * 512],
                    z2st[:],
                )

        nc.gpsimd.collective_compute(
            "ReduceScatter",
            ALU.add,
            ins=[z2_dram[:, :]],
            outs=[z2own[:, :]],
            replica_groups=RG,
        )

        # ================= h2 = tanh(z2own); L3: pts = h2 @ W3 + b3 ========
        z2o = work.tile([GPC, HID], F32, tag="z2o")
        nc.sync.dma_start(z2o[:], z2own[:, :])
        h2 = work.tile([GPC, HID], BF16, tag="h2")
        nc.scalar.activation(h2[:], z2o[:], AF.Tanh)

        h2T = work.tile([128, HID // 128, GPC], BF16, tag="h2T")
        for kc in range(HID // 128):
            pt = pstr.tile([128, GPC], F32, tag="ptr2")
            nc.tensor.transpose(
                pt[:], h2[:, kc * 128 : (kc + 1) * 128], idb_sb[0:GPC, 0:GPC]
            )
            nc.vector.tensor_copy(h2T[:, kc, :], pt[:])

        pp = psmm.tile([GPC, 256], F32, tag="pp")
        for kc in range(HID // 128):
            nc.tensor.matmul(
                pp[:],
                lhsT=h2T[:, kc, :],
                rhs=w3_sb[:, kc, :],
                start=(kc == 0),
                stop=False,
            )
        nc.tensor.matmul(
            pp[:], lhsT=ones_sb[:, 0:GPC], rhs=w3a_sb[:], start=False, stop=True
        )
        pts_sb = work.tile([GPC, 2 * NPG], F32, tag="ptssb")
        nc.vector.tensor_copy(pts_sb[:], pp[:, 0 : 2 * NPG])
        # kernel output (write-only)
        nc.sync.dma_start(
            pts_p[:, :].rearrange("(g i) c -> g (i c)", g=GPC), pts_sb[:]
        )
        # bounce through tracked DRAM tile to reload transposed
        pts_dt = dramp.tile([GPC, 2 * NPG], F32)
        nc.sync.dma_start(pts_dt[:], pts_sb[:])
        ptsT_sb = const.tile([2, NPC], F32)
        nc.sync.dma_start(
            ptsT_sb[:].rearrange("c (g i) -> c g i", g=GPC),
            pts_dt[:].rearrange("g (i c) -> c g i", c=2),
        )

        # ================= ECT layer 2 + normalization =====================
        e2_sb, mx2 = ect_layer(ptsT_sb, s2_sb, "ect2")
        rinv2 = small.tile([GPC, 1], F32, tag="rinv2")
        nc.vector.reciprocal(rinv2[:], mx2[:])
        dec_sb = work.tile([GPC, D], F32, tag="dec")
        nc.vector.tensor_scalar_mul(dec_sb[:], e2_sb[:, 0:D], rinv2[:])
        nc.sync.dma_start(dec_p[:, :], dec_sb[:])

    return nc


def _get_nc(stage="full"):
    if stage not in _CACHE:
        _CACHE[stage] = _build(stage)
    return _CACHE[stage]


def _prep_inputs(x, batch_idx, V, lin, W1, b1, W2, b2, W3, b3):
    import ml_dtypes

    bf16 = ml_dtypes.bfloat16
    x = np.asarray(x, np.float32)
    batch_idx = np.asarray(batch_idx)
    V = np.ascontiguousarray(np.asarray(V, np.float32))
    lin = np.asarray(lin, np.float32)
    W1 = np.asarray(W1, np.float32)
    b1 = np.asarray(b1, np.float32)
    W2 = np.asarray(W2, np.float32)
    b2 = np.asarray(b2, np.float32)
    W3 = np.asarray(W3, np.float32)
    b3 = np.asarray(b3, np.float32)

    # sort nodes by graph id (stable) so each core gets contiguous graphs
    order = np.argsort(batch_idx, kind="stable")
    x_sorted = x[order]
    bs = np.asarray(batch_idx)[order].astype(np.int64)

    # indicator matrices for the per-graph segment sums
    gid = np.arange(NCORES * GPC).reshape(NCORES, GPC)
    bs_r = bs.reshape(NCORES, NT, 128)
    S1 = (bs_r[:, :, :, None] == gid[:, None, None, :]).astype(bf16)
    # second ECT layer always uses uniform 100-node graphs
    node_g = (np.arange(NPC) // NPG).reshape(NT, 128)
    S2 = (node_g[:, :, None] == np.arange(GPC)[None, None, :]).astype(bf16)
    S2 = np.broadcast_to(S2, (NCORES, NT, 128, GPC))

    biasrep = np.ascontiguousarray(
        np.broadcast_to((SCALE * lin)[None, :], (128, J)).astype(np.float32)
    )
    idf32 = np.eye(128, dtype=np.float32)
    idbf16 = np.eye(128, dtype=bf16)

    w3a = np.zeros((HID + 1, 256), np.float32)
    w3a[:HID, :200] = W3
    w3a[HID, :200] = b3
    w3a = w3a.astype(bf16)

    in_maps = []
    for c in range(NCORES):
        xT = np.ascontiguousarray(x_sorted[c * NPC : (c + 1) * NPC].T)
        w1s = np.concatenate(
            [W1[:, c * WCOL : (c + 1) * WCOL], b1[None, c * WCOL : (c + 1) * WCOL]],
            axis=0,
        ).astype(bf16)
        w2s = np.concatenate(
            [W2[c * WCOL : (c + 1) * WCOL, :], (b2 / NCORES)[None, :]], axis=0
        ).astype(bf16)
        in_maps.append(
            {
                "xt": xT,
                "v": V,
                "biasrep": biasrep,
                "s1": np.ascontiguousarray(S1[c]),
                "s2": np.ascontiguousarray(S2[c]),
                "w1s": np.ascontiguousarray(w1s),
                "w2s": np.ascontiguousarray(w2s),
                "w3a": w3a,
                "idf32": idf32,
                "idbf16": idbf16,
            }
        )
    return in_maps


def run(stage="full", trace=False, **inputs):
    from concourse.bass_utils import run_bass_kernel_spmd

    nc = _get_nc(stage)
    in_maps = _prep_inputs(**inputs)
    res = run_bass_kernel_spmd(
        nc, in_maps, core_ids=list(range(NCORES)), trace=trace
    )
    return res


def kernel(**inputs):
    res = run(stage="full", trace=False, **inputs)
    decoded = np.concatenate(
        [res.results[c]["decoded"] for c in range(NCORES)], axis=0
    ).reshape(B, J, T)
    pts = np.concatenate(
        [res.results[c]["pts"] for c in range(NCORES)], axis=0
    )
    return decoded, pts


# revision 5
# speedup vs baseline: 3.5307x; 3.5307x over previous
"""Trainium2 Bass kernel for nn_Autoencoder_65120294142543 (ECT autoencoder).

Sharding (8 NeuronCores, one TRN2 chip):
  - ECT layers: data-parallel over graphs (32 graphs = 3200 nodes per core).
  - MLP: tensor-parallel. W1 column-sharded [4096, 512/core] (bf16),
    W2 row-sharded [512/core, 4096] (bf16), W3 replicated (bf16).
    AllGather of the (augmented) ECT output before L1; ReduceScatter of the
    L2 partial sums (by graph) after L2.
  - Normalization e/max(e) is folded into the MLP via an augmented
    contraction row: Z1 = e@W1 + mx*b1, h1 = tanh(Z1 * (1/mx)).
"""

import numpy as np

# ---------------- problem constants (hardcoded per the task spec) ----------
B = 256          # graphs
NPG = 100        # nodes per graph
T = 64           # num directions (thetas)
J = 64           # bump steps (thresholds)
D = J * T        # 4096 = flattened ECT size = MLP input dim
HID = 4096
NCORES = 8
GPC = B // NCORES          # 32 graphs per core
NPC = GPC * NPG            # 3200 nodes per core
NT = NPC // 128            # 25 node tiles of 128
JGS = 8                    # js per sigma/segsum group
NJG = J // JGS             # 8 groups
WCOL = HID // NCORES       # 512 W1 columns / W2 rows per core
SCALE = 500.0
NB2 = HID // 512           # 8 n-blocks for L2 output

_CACHE = {}


def _patch_tile_drain():
    """The walrus build deployed here supports fewer sem-waits per CTRL
    instruction than Tile's kernel-tail drain accumulates.  Split the
    drain's waits into individual wait_ge instructions."""
    from concourse import tile
    from concourse.tile import ScopedClock

    if getattr(tile.TileContext, "_drain_patched", False):
        return

    def _drain_and_barrier(self, tick_clock, wait_clock):
        drain_inst = self.nc.sync.drain()
        wait_clock.add_sem_waits(
            drain_inst.ins, ScopedClock({None: tick_clock.global_clock})
        )
        si = drain_inst.ins.sync_info
        if si is not None and si.on_wait and len(si.on_wait) > 1:
            waits = list(si.on_wait)
            si.on_wait = []
            by_name = {h.name: h for h in self.sems.allocated().values()}
            for w in waits:
                self.nc.sync.wait_ge(by_name[w.ant_name], w.wait_value)
        self.nc.all_engine_barrier()
        popped = self.nc._tile_sem_poison_stack.pop()
        assert popped is self._sem_poison
        self.nc.clear_and_free_semaphores(list(self.sems.allocated().values()))
        self.nc.all_engine_barrier()

    tile.TileContext._drain_and_barrier = _drain_and_barrier
    tile.TileContext._drain_patched = True


def _split_waits(nc, limit=1):
    """The deployed walrus supports only `limit` sem-waits per engine
    instruction.  Hoist extra waits onto NoOp carriers inserted before."""
    from concourse import mybir

    engines = {
        mybir.EngineType.PE,
        mybir.EngineType.Activation,
        mybir.EngineType.DVE,
        mybir.EngineType.Pool,
        mybir.EngineType.SP,
    }
    k = 0
    for bb in nc.main_func.blocks:
        insts = bb.instructions
        i = 0
        while i < len(insts):
            ins = insts[i]
            si = ins.sync_info
            if (
                si is not None
                and si.on_wait
                and len(si.on_wait) > limit
                and ins.engine in engines
            ):
                waits = list(si.on_wait)
                si.on_wait = waits[:limit]
                carriers = []
                for w in waits[limit:]:
                    nop = mybir.InstNoOp(
                        name=f"{ins.name}-sw{k}", ins=[], outs=[], engine=ins.engine
                    )
                    nop.sync_info = mybir.SyncInfo(on_wait=[w], on_update=[])
                    carriers.append(nop)
                    k += 1
                for j, nop in enumerate(carriers):
                    insts.insert(i + j, nop)
                i += len(carriers)
            i += 1
    return k


def _build(stage="full"):
    from contextlib import ExitStack
    from concourse import bass, tile, mybir

    _patch_tile_drain()

    F32 = mybir.dt.float32
    BF16 = mybir.dt.bfloat16
    AF = mybir.ActivationFunctionType
    AX = mybir.AxisListType
    ALU = mybir.AluOpType

    nc = bass.Bass(target_bir_lowering=False)

    # ---- per-core external inputs -----------------------------------------
    xT_p = nc.dram_tensor("xt", [2, NPC], F32, kind="ExternalInput")
    v_p = nc.dram_tensor("v", [2, T], F32, kind="ExternalInput")
    bias_p = nc.dram_tensor("biasrep", [128, J], F32, kind="ExternalInput")
    s1_p = nc.dram_tensor("s1", [NT, 128, GPC], BF16, kind="ExternalInput")
    s2_p = nc.dram_tensor("s2", [NT, 128, GPC], BF16, kind="ExternalInput")
    w1_p = nc.dram_tensor("w1s", [HID + 1, WCOL], BF16, kind="ExternalInput")
    w2_p = nc.dram_tensor("w2s", [WCOL + 1, HID], BF16, kind="ExternalInput")
    w3_p = nc.dram_tensor("w3a", [HID + 1, 256], BF16, kind="ExternalInput")
    idf_p = nc.dram_tensor("idf32", [128, 128], F32, kind="ExternalInput")
    idb_p = nc.dram_tensor("idbf16", [128, 128], BF16, kind="ExternalInput")

    # ---- per-core external outputs ----------------------------------------
    dec_p = nc.dram_tensor("decoded", [GPC, D], F32, kind="ExternalOutput")
    pts_p = nc.dram_tensor("pts", [NPC, 2], F32, kind="ExternalOutput")
    dbg_p = None
    if stage == "ect1":
        dbg_p = nc.dram_tensor("dbg_e", [GPC, D + 8], F32, kind="ExternalOutput")

    # ---- internal DRAM (collective bounce buffers) ------------------------
    ag_in = nc.dram_tensor("ag_in", [GPC, D + 8], F32)
    ag_out = nc.dram_tensor("ag_out", [B, D + 8], F32, addr_space="Shared")
    z2_dram = nc.dram_tensor("z2part", [B, HID], F32)
    z2own = nc.dram_tensor("z2own", [GPC, HID], F32, addr_space="Shared")

    RG = [list(range(NCORES))]

    with ExitStack() as ctx:
        tc = ctx.enter_context(tile.TileContext(nc))
        const = ctx.enter_context(tc.tile_pool(name="const", bufs=1))
        work = ctx.enter_context(tc.tile_pool(name="work", bufs=2))
        sigp = ctx.enter_context(tc.tile_pool(name="sigp", bufs=2))
        small = ctx.enter_context(tc.tile_pool(name="small", bufs=2))
        dramp = ctx.enter_context(tc.tile_pool(name="dramp", bufs=1, space="DRAM"))
        # PSUM budget is 8 banks of [128, 2KB]; keep static tag footprint <= 6
        pse = ctx.enter_context(tc.tile_pool(name="pse", bufs=2, space="PSUM"))
        pstr = ctx.enter_context(tc.tile_pool(name="pstr", bufs=2, space="PSUM"))
        psmm = ctx.enter_context(tc.tile_pool(name="psmm", bufs=2, space="PSUM"))

        # ---------- constants into SBUF (DMA'd once; overlap with compute)
        v_sb = const.tile([2, T], F32)
        nc.sync.dma_start(v_sb[:], v_p[:, :])
        bias_sb = const.tile([128, J], F32)
        nc.sync.dma_start(bias_sb[:], bias_p[:, :])
        s1_sb = const.tile([128, NT, GPC], BF16)
        nc.sync.dma_start(s1_sb[:], s1_p[:, :, :].transpose([1, 0, 2]))
        s2_sb = const.tile([128, NT, GPC], BF16)
        nc.sync.dma_start(s2_sb[:], s2_p[:, :, :].transpose([1, 0, 2]))
        idf_sb = const.tile([128, 128], F32)
        nc.sync.dma_start(idf_sb[:], idf_p[:, :])
        idb_sb = const.tile([128, 128], BF16)
        nc.sync.dma_start(idb_sb[:], idb_p[:, :])
        ones_sb = const.tile([1, 128], BF16)
        nc.vector.memset(ones_sb[:], 1.0)

        w1_sb = const.tile([128, HID // 128, WCOL], BF16)
        nc.sync.dma_start(
            w1_sb[:], w1_p[0:HID, :].rearrange("(a p) n -> p a n", p=128)
        )
        w1a_sb = const.tile([1, WCOL], BF16)
        nc.sync.dma_start(w1a_sb[:], w1_p[HID : HID + 1, :])

        w2_sb = const.tile([128, WCOL // 128, HID], BF16)
        nc.sync.dma_start(
            w2_sb[:], w2_p[0:WCOL, :].rearrange("(a p) n -> p a n", p=128)
        )
        w2a_sb = const.tile([1, HID], BF16)
        nc.sync.dma_start(w2a_sb[:], w2_p[WCOL : WCOL + 1, :])

        w3_sb = const.tile([128, HID // 128, 256], BF16)
        nc.sync.dma_start(
            w3_sb[:], w3_p[0:HID, :].rearrange("(a p) n -> p a n", p=128)
        )
        w3a_sb = const.tile([1, 256], BF16)
        nc.sync.dma_start(w3a_sb[:], w3_p[HID : HID + 1, :])

        xT_sb = const.tile([2, NPC], F32)
        nc.sync.dma_start(xT_sb[:], xT_p[:, :])

        # ---------- one ECT layer: heights -> sigmoids -> per-graph sums ---
        def ect_layer(srcT_sb, S_sb, tag):
            # nh[node, t] = src[node, :] @ V   (nodes on partitions, tiled)
            nh_sb = work.tile([128, NT, T], F32, tag="nh")
            for tau in range(NT):
                pm = pstr.tile([128, T], F32, tag="ptr")
                nc.tensor.matmul(
                    pm[:],
                    lhsT=srcT_sb[:, tau * 128 : (tau + 1) * 128],
                    rhs=v_sb[:],
                    start=True,
                    stop=True,
                )
                nc.vector.tensor_copy(nh_sb[:, tau, :], pm[:])

            # e[g, j*64+t] accumulated in groups of JGS thresholds
            e_sb = work.tile([GPC, D + 8], F32, tag="e")
            nc.vector.memset(e_sb[:, D : D + 8], 0.0)
            for jg in range(NJG):
                sig = sigp.tile([128, NT, JGS, T], BF16, tag="sig")
                for jj in range(JGS):
                    j = jg * JGS + jj
                    nc.scalar.activation(
                        sig[:, :, jj, :],
                        nh_sb[:, :, :],
                        AF.Sigmoid,
                        bias=bias_sb[:, j : j + 1],
                        scale=-SCALE,
                    )
                pe = pse.tile([GPC, JGS * T], F32, tag="pse")
                for tau in range(NT):
                    nc.tensor.matmul(
                        pe[:],
                        lhsT=S_sb[:, tau, :],
                        rhs=sig[:, tau, :, :].rearrange("p a b -> p (a b)"),
                        start=(tau == 0),
                        stop=(tau == NT - 1),
                    )
                nc.vector.tensor_copy(
                    e_sb[:, jg * JGS * T : (jg + 1) * JGS * T], pe[:]
                )
            # per-graph max -> augmented column D
            mx = small.tile([GPC, 1], F32, tag="mx")
            nc.vector.reduce_max(mx[:], e_sb[:, 0:D], axis=AX.X)
            nc.vector.tensor_copy(e_sb[:, D : D + 1], mx[:])
            return e_sb, mx

        # ================= ECT layer 1 =====================================
        e1_sb, _mx1 = ect_layer(xT_sb, s1_sb, "ect1")
        nc.sync.dma_start(ag_in[:, :], e1_sb[:])

        if stage == "ect1":
            nc.sync.dma_start(dbg_p[:, :], e1_sb[:])
            ctx.close()
            _split_waits(nc)
            return nc

        nc.gpsimd.collective_compute(
            "AllGather",
            ALU.bypass,
            ins=[ag_in[:, :]],
            outs=[ag_out[:, :]],
            replica_groups=RG,
        )

        # ================= MLP L1 (tensor-parallel over W1 columns) ========
        # lhsT chunks: transpose e_full [256, 4096] -> eT [4096, 256] in bf16
        h1_sb = []  # per g-block [128, WCOL] bf16
        for gb in range(2):
            gsl = slice(gb * 128, (gb + 1) * 128)
            z1 = psmm.tile([128, WCOL], F32, tag="zmm")
            for kc in range(HID // 128):
                eg = small.tile([128, 128], F32, tag="eg")
                nc.sync.dma_start(
                    eg[:], ag_out[gsl, kc * 128 : (kc + 1) * 128]
                )
                pt = pstr.tile([128, 128], F32, tag="ptr")
                nc.tensor.transpose(pt[:], eg[:], idf_sb[:])
                eT = small.tile([128, 128], BF16, tag="eT")
                nc.vector.tensor_copy(eT[:], pt[:])
                nc.tensor.matmul(
                    z1[:],
                    lhsT=eT[:],
                    rhs=w1_sb[:, kc, :],
                    start=(kc == 0),
                    stop=False,
                )
            # augmented row: mx (per graph) * b1
            mxT = small.tile([1, 128], F32, tag="mxT")
            nc.sync.dma_start(
                mxT[:], ag_out[gsl, D : D + 1].transpose([1, 0])
            )
            mxTb = small.tile([1, 128], BF16, tag="mxTb")
            nc.vector.tensor_copy(mxTb[:], mxT[:])
            nc.tensor.matmul(
                z1[:], lhsT=mxTb[:], rhs=w1a_sb[:], start=False, stop=True
            )
            # h1 = tanh(z1 / mx)
            mxg = small.tile([128, 1], F32, tag="mxg")
            nc.sync.dma_start(mxg[:], ag_out[gsl, D : D + 1])
            rinv = small.tile([128, 1], F32, tag="rinv")
            nc.vector.reciprocal(rinv[:], mxg[:])
            h1 = work.tile([128, WCOL], BF16, tag="h1")
            nc.scalar.activation(h1[:], z1[:], AF.Tanh, scale=rinv[:])
            h1_sb.append(h1)

        # ================= MLP L2 (row-sharded W2, partial sums) ===========
        for gb in range(2):
            h1T = work.tile([128, WCOL // 128, 128], BF16, tag="h1T")
            for kc in range(WCOL // 128):
                pt = pstr.tile([128, 128], F32, tag="ptr")
                nc.tensor.transpose(
                    pt[:], h1_sb[gb][:, kc * 128 : (kc + 1) * 128], idb_sb[:]
                )
                nc.vector.tensor_copy(h1T[:, kc, :], pt[:])
            for nb in range(NB2):
                z2 = psmm.tile([128, 512], F32, tag="zmm")
                for kc in range(WCOL // 128):
                    nc.tensor.matmul(
                        z2[:],
                        lhsT=h1T[:, kc, :],
                        rhs=w2_sb[:, kc, nb * 512 : (nb + 1) * 512],
                        start=(kc == 0),
                        stop=False,
                    )
                nc.tensor.matmul(
                    z2[:],
                    lhsT=ones_sb[:],
                    rhs=w2a_sb[:, nb * 512 : (nb + 1) * 512],
                    start=False,
                    stop=True,
                )
                z2st = small.tile([128, 512], F32, tag="z2st")
                nc.vector.tensor_copy(z2st[:], z2[:])
                nc.sync.dma_start(
                    z2_dram[gb * 128 : (gb + 1) * 128, nb * 512 : (nb + 1) * 512],
                    z2st[:],
                )

        nc.gpsimd.collective_compute(
            "ReduceScatter",
            ALU.add,
            ins=[z2_dram[:, :]],
            outs=[z2own[:, :]],
            replica_groups=RG,
        )

        # ================= h2 = tanh(z2own); L3: pts = h2 @ W3 + b3 ========
        z2o = work.tile([GPC, HID], F32, tag="z2o")
        nc.sync.dma_start(z2o[:], z2own[:, :])
        h2 = work.tile([GPC, HID], BF16, tag="h2")
        nc.scalar.activation(h2[:], z2o[:], AF.Tanh)

        h2T = work.tile([128, HID // 128, GPC], BF16, tag="h2T")
        for kc in range(HID // 128):
            pt = pstr.tile([128, GPC], F32, tag="ptr")
            nc.tensor.transpose(
                pt[:], h2[:, kc * 128 : (kc + 1) * 128], idb_sb[0:GPC, 0:GPC]
            )
            nc.vector.tensor_copy(h2T[:, kc, :], pt[:])

        pp = pse.tile([GPC, 256], F32, tag="pse")
        for kc in range(HID // 128):
            nc.tensor.matmul(
                pp[:],
                lhsT=h2T[:, kc, :],
                rhs=w3_sb[:, kc, :],
                start=(kc == 0),
                stop=False,
            )
        nc.tensor.matmul(
            pp[:], lhsT=ones_sb[:, 0:GPC], rhs=w3a_sb[:], start=False, stop=True
        )
        pts_sb = work.tile([GPC, 2 * NPG], F32, tag="ptssb")
        nc.vector.tensor_copy(pts_sb[:], pp[:, 0 : 2 * NPG])
        # kernel output (write-only)
        nc.sync.dma_start(
            pts_p[:, :].rearrange("(g i) c -> g (i c)", g=GPC), pts_sb[:]
        )
        # bounce through tracked DRAM tile to reload transposed
        pts_dt = dramp.tile([GPC, 2 * NPG], F32)
        nc.sync.dma_start(pts_dt[:], pts_sb[:])
        ptsT_sb = const.tile([2, NPC], F32)
        nc.sync.dma_start(
            ptsT_sb[:].rearrange("c (g i) -> c g i", g=GPC),
            pts_dt[:].rearrange("g (i c) -> c g i", c=2),
        )

        # ================= ECT layer 2 + normalization =====================
        e2_sb, mx2 = ect_layer(ptsT_sb, s2_sb, "ect2")
        rinv2 = small.tile([GPC, 1], F32, tag="rinv2")
        nc.vector.reciprocal(rinv2[:], mx2[:])
        dec_sb = work.tile([GPC, D], F32, tag="dec")
        nc.vector.tensor_scalar_mul(dec_sb[:], e2_sb[:, 0:D], rinv2[:])
        nc.sync.dma_start(dec_p[:, :], dec_sb[:])

    _split_waits(nc)
    return nc


def _get_nc(stage="full"):
    if stage not in _CACHE:
        _CACHE[stage] = _build(stage)
    return _CACHE[stage]


def _prep_inputs(x, batch_idx, V, lin, W1, b1, W2, b2, W3, b3):
    import ml_dtypes

    bf16 = ml_dtypes.bfloat16
    x = np.asarray(x, np.float32)
    batch_idx = np.asarray(batch_idx)
    V = np.ascontiguousarray(np.asarray(V, np.float32))
    lin = np.asarray(lin, np.float32)
    W1 = np.asarray(W1, np.float32)
    b1 = np.asarray(b1, np.float32)
    W2 = np.asarray(W2, np.float32)
    b2 = np.asarray(b2, np.float32)
    W3 = np.asarray(W3, np.float32)
    b3 = np.asarray(b3, np.float32)

    # sort nodes by graph id (stable) so each core gets contiguous graphs
    order = np.argsort(batch_idx, kind="stable")
    x_sorted = x[order]
    bs = np.asarray(batch_idx)[order].astype(np.int64)

    # indicator matrices for the per-graph segment sums
    gid = np.arange(NCORES * GPC).reshape(NCORES, GPC)
    bs_r = bs.reshape(NCORES, NT, 128)
    S1 = (bs_r[:, :, :, None] == gid[:, None, None, :]).astype(bf16)
    # second ECT layer always uses uniform 100-node graphs
    node_g = (np.arange(NPC) // NPG).reshape(NT, 128)
    S2 = (node_g[:, :, None] == np.arange(GPC)[None, None, :]).astype(bf16)
    S2 = np.broadcast_to(S2, (NCORES, NT, 128, GPC))

    biasrep = np.ascontiguousarray(
        np.broadcast_to((SCALE * lin)[None, :], (128, J)).astype(np.float32)
    )
    idf32 = np.eye(128, dtype=np.float32)
    idbf16 = np.eye(128, dtype=bf16)

    w3a = np.zeros((HID + 1, 256), np.float32)
    w3a[:HID, :200] = W3
    w3a[HID, :200] = b3
    w3a = w3a.astype(bf16)

    in_maps = []
    for c in range(NCORES):
        xT = np.ascontiguousarray(x_sorted[c * NPC : (c + 1) * NPC].T)
        w1s = np.concatenate(
            [W1[:, c * WCOL : (c + 1) * WCOL], b1[None, c * WCOL : (c + 1) * WCOL]],
            axis=0,
        ).astype(bf16)
        w2s = np.concatenate(
            [W2[c * WCOL : (c + 1) * WCOL, :], (b2 / NCORES)[None, :]], axis=0
        ).astype(bf16)
        in_maps.append(
            {
                "xt": xT,
                "v": V,
                "biasrep": biasrep,
                "s1": np.ascontiguousarray(S1[c]),
                "s2": np.ascontiguousarray(S2[c]),
                "w1s": np.ascontiguousarray(w1s),
                "w2s": np.ascontiguousarray(w2s),
                "w3a": w3a,
                "idf32": idf32,
                "idbf16": idbf16,
            }
        )
    return in_maps


def run(stage="full", trace=False, **inputs):
    from concourse.bass_utils import run_bass_kernel_spmd

    nc = _get_nc(stage)
    in_maps = _prep_inputs(**inputs)
    res = run_bass_kernel_spmd(
        nc, in_maps, core_ids=list(range(NCORES)), trace=trace
    )
    return res


def kernel(**inputs):
    res = run(stage="full", trace=False, **inputs)
    decoded = np.concatenate(
        [res.results[c]["decoded"] for c in range(NCORES)], axis=0
    ).reshape(B, J, T)
    pts = np.concatenate(
        [res.results[c]["pts"] for c in range(NCORES)], axis=0
    )
    return decoded, pts
